# revision 15
# baseline (speedup 1.0000x reference)
"""Trainium2 Bass kernel for a 2-layer GraphSAGE (mean aggregation) GNN.

Contract: kernel(**inputs) takes the FULL inputs from setup_inputs() and
returns the FULL [50000, 128] float32 output, distributing work across 8
NeuronCores internally.

Strategy (self-contained; constants hardcoded for N=50000, E=600000, F=128):
  - Shard nodes (and their incoming edges) by dst range: core c owns nodes
    [c*6250, (c+1)*6250).
  - All message/weight data is bf16 on device (psum accumulation stays f32):
    halves gather + AllGather + DRAM traffic, 4x PE matmul throughput, 2x DVE.
  - Per core, group edges by 128-wide dst blocks; within a block split by
    src table half (dma_gather indices are int16, so each gather table is
    split into two <32768-row halves); pad each (block, table) edge list to a
    multiple of 128 (chunk) with dummy edges (idx 0, dstloc -1).
  - Gather x[src] rows (256B bf16) from HBM with gpsimd.dma_gather, batched
    over superbatches of blocks, one desc-gen call per table side per
    superbatch (dynamic_dma_scratch_size=65536 so up to ~3.5k idxs/call),
    rotated over all 4 SWDGE queues.
  - Segment-mean via PE without any transpose: onehot[e, v] =
    (dstloc[e]==v) * rdeg[e] built in ONE DVE tensor_scalar (is_equal, mult)
    against an iota row; psum_aggT[f, v] += g_chunk[e, f]^T-contracted
    onehot — i.e. matmul(lhsT=g_chunk, rhs=onehot). rdeg = 1/max(deg,1) is
    precomputed per edge slot on host, so the mean needs no extra ops.
  - hT = W_l^T @ aggT + W_r^T @ xT (+bias, relu on ACT). xT comes in
    host-pre-transposed; hT stays resident in SBUF for layer 2's root term.
    One PE transpose per block converts hT to node-major h for the
    AllGather/gather table. Layer-2 output is written transposed
    (out_shardT) and un-transposed on host.
  - The h shard is AllGathered between layers in ONE collective (measured
    cost on this stack is ~184us fixed + ~31us/MB of output per collective,
    so one large AllGather beats two smaller overlapped ones). Layer 2
    repeats the pipeline on h_full with its own (block, table) edge
    grouping, tables split at TAB2 (cores 0-3 vs 4-7) for int16 indices.
"""
import sys

sys.path.insert(0, "/opt/trn_rl_repo")

from contextlib import ExitStack

import numpy as np

N = 50000
E = 600000
F = 128
NC = 8
NPC = N // NC          # 6250 nodes per core
NB = (NPC + 127) // 128  # 49 dst blocks per core
NPCP = NB * 128        # 6272 padded nodes per core
NP = NC * NPCP         # 50176 padded total
TAB1 = N // 2          # 25000: layer-1 lo/hi table split
TAB2 = NP // 2         # 25088: layer-2 split (= 4 cores * 6272)
SBS = 4                # blocks per gather superbatch (= agg psum bufs)
NBA = 25               # L1 blocks whose h rows go in the first AllGather
HA_ROWS = NBA * 128    # 3200 rows/core in h_shard_a
HB_ROWS = (NB - NBA) * 128  # 3072 rows/core in h_shard_b
GMAX = 512             # max idxs per dma_gather call (512 pipelines ~2x better than 1024)
DMA_SCRATCH = 16384    # dynamic_dma_scratch_size (bytes): desc FIFO capacity

_cache = {}


def _bf16():
    from ml_dtypes import bfloat16

    return bfloat16


def _ceil_div(a, b):
    return -(-a // b)


def _host_prep(x, edge_index):
    """Build per-core padded gather/index/dstloc/rdeg arrays (host-side
    index bookkeeping, like CSR construction)."""
    src = np.asarray(edge_index[0], dtype=np.int64)
    dst = np.asarray(edge_index[1], dtype=np.int64)
    bf16 = _bf16()
    core = dst // NPC
    blk = (dst % NPC) >> 7
    dloc = (dst % NPC) & 127

    # per-core degree, then per-edge 1/max(deg,1)
    deg = np.zeros((NC, NPC), np.float64)
    for c in range(NC):
        deg[c] = np.bincount(dst[core == c] % NPC, minlength=NPC)
    rdeg_edge = (1.0 / np.maximum(deg, 1.0))[core, dst % NPC].astype(np.float32)

    def wrap(a):
        n = a.shape[1]
        w = np.ascontiguousarray(a.reshape(NC, n // 16, 16).transpose(0, 2, 1))
        return np.tile(w, (1, 8, 1))  # [NC, 128, n//16]

    def prep_layer(tab, rowid):
        """tab[e] in {0,1}: which gather table; rowid[e]: row within table."""
        key = (core * NB + blk) * 2 + tab
        order = np.lexsort((rowid, key))
        s_row = rowid[order]
        s_dloc = dloc[order]
        s_rdeg = rdeg_edge[order]
        s_key = key[order]
        bounds = np.searchsorted(s_key, np.arange(NC * NB * 2 + 1))
        cnt = (bounds[1:] - bounds[:-1]).reshape(NC, NB, 2)
        chunks = _ceil_div(cnt, 128)
        c_lo = chunks[:, :, 0].max(axis=0)
        c_hi = chunks[:, :, 1].max(axis=0)
        nch = c_lo + c_hi
        K_lo = int(c_lo.sum()) * 128
        K_hi = int(c_hi.sum()) * 128
        NCH = int(nch.sum())
        idx_lo = np.zeros((NC, K_lo), np.int16)
        idx_hi = np.zeros((NC, K_hi), np.int16)
        dcols = np.full((NC, NCH * 128), -1.0, np.float32)
        rcols = np.zeros((NC, NCH * 128), np.float32)
        for c in range(NC):
            off_lo = off_hi = off_q = 0
            for b in range(NB):
                i0 = bounds[(c * NB + b) * 2]
                i1 = bounds[(c * NB + b) * 2 + 1]
                i2 = bounds[(c * NB + b) * 2 + 2]
                nlo = i1 - i0
                nhi = i2 - i1
                idx_lo[c, off_lo:off_lo + nlo] = s_row[i0:i1]
                idx_hi[c, off_hi:off_hi + nhi] = s_row[i1:i2]
                dcols[c, off_q:off_q + nlo] = s_dloc[i0:i1]
                dcols[c, off_q + c_lo[b] * 128:off_q + c_lo[b] * 128 + nhi] = s_dloc[i1:i2]
                rcols[c, off_q:off_q + nlo] = s_rdeg[i0:i1]
                rcols[c, off_q + c_lo[b] * 128:off_q + c_lo[b] * 128 + nhi] = s_rdeg[i1:i2]
                off_lo += c_lo[b] * 128
                off_hi += c_hi[b] * 128
                off_q += nch[b] * 128
        dl = np.ascontiguousarray(
            dcols.reshape(NC, NCH, 128).transpose(0, 2, 1))
        rl = np.ascontiguousarray(
            rcols.reshape(NC, NCH, 128).transpose(0, 2, 1))
        return dict(c_lo=tuple(int(v) for v in c_lo), c_hi=tuple(int(v) for v in c_hi),
                    idx_lo=wrap(idx_lo), idx_hi=wrap(idx_hi), dl=dl, rl=rl,
                    K_lo=K_lo, K_hi=K_hi, NCH=NCH)

    # layer 1: split x table at row TAB1
    t1 = (src >= TAB1).astype(np.int64)
    r1 = np.where(t1 == 0, src, src - TAB1)
    L1 = prep_layer(t1, r1)
    # layer 2: h is all-gathered into one [NP, F] table ordered by core;
    # int16 gather rows force a lo/hi table split at TAB2 (cores 0-3 / 4-7)
    rc = src % NPC
    cc = src // NPC
    g2 = cc * NPCP + rc
    t2 = (g2 >= TAB2).astype(np.int64)
    r2 = np.where(t2 == 0, g2, g2 - TAB2)
    L2 = prep_layer(t2, r2)

    return {
        "L1": L1, "L2": L2,
        "dstloc": np.concatenate([L1["dl"], L2["dl"]], axis=2),
        "rdeg": np.concatenate([L1["rl"], L2["rl"]], axis=2),
        "NCH": L1["NCH"] + L2["NCH"],
    }


def _build(L1, L2, NCH, loop_reps=0, stage="full", one_queue=False, gmax=None,
           pg_bufs=2, ps_bufs=4, sp=True):
    from concourse import bacc, tile
    from concourse.bass import mybir

    f32 = mybir.dt.float32
    bf16 = mybir.dt.bfloat16
    i16 = mybir.dt.int16
    AF = mybir.ActivationFunctionType
    OP = mybir.AluOpType

    nc = bacc.Bacc("TRN2", target_bir_lowering=False, debug=False, num_devices=NC,
                   num_swdge_queues=4, dynamic_dma_scratch_size=DMA_SCRATCH)

    x_tab = nc.declare_dram_parameter("x_tab", [N, F], bf16, isOutput=False)
    x_selfT = nc.declare_dram_parameter("x_selfT", [128, NPCP], bf16, isOutput=False)
    d_idx1_lo = nc.declare_dram_parameter("idx1_lo", [128, L1["K_lo"] // 16], i16, isOutput=False)
    d_idx1_hi = nc.declare_dram_parameter("idx1_hi", [128, L1["K_hi"] // 16], i16, isOutput=False)
    d_idx2_lo = nc.declare_dram_parameter("idx2_lo", [128, L2["K_lo"] // 16], i16, isOutput=False)
    d_idx2_hi = nc.declare_dram_parameter("idx2_hi", [128, L2["K_hi"] // 16], i16, isOutput=False)
    d_dstloc = nc.declare_dram_parameter("dstloc", [128, NCH], f32, isOutput=False)
    d_rdeg = nc.declare_dram_parameter("rdeg", [128, NCH], f32, isOutput=False)
    d_wl1 = nc.declare_dram_parameter("wl1", [F, F], bf16, isOutput=False)
    d_wr1 = nc.declare_dram_parameter("wr1", [F, F], bf16, isOutput=False)
    d_wl2 = nc.declare_dram_parameter("wl2", [F, F], bf16, isOutput=False)
    d_wr2 = nc.declare_dram_parameter("wr2", [F, F], bf16, isOutput=False)
    d_b1 = nc.declare_dram_parameter("b1c", [128, 1], f32, isOutput=False)
    d_b2 = nc.declare_dram_parameter("b2c", [128, 1], f32, isOutput=False)
    d_iota = nc.declare_dram_parameter("iota", [128, 128], bf16, isOutput=False)
    d_ident = nc.declare_dram_parameter("ident", [128, 128], bf16, isOutput=False)
    out_shardT = nc.declare_dram_parameter("out_shardT", [128, NPCP], f32, isOutput=True)

    # single h shard + one AllGather: measured collective cost is
    # latency-dominated (~184us fixed per collective + ~31us/MB), so one
    # large AllGather beats two smaller overlapped ones.
    h_shard = nc.dram_tensor("h_shard", [NPCP, F], bf16)
    h_full = nc.dram_tensor("h_full", [NC * NPCP, F], bf16, addr_space="Shared")

    with tile.TileContext(nc) as tc, ExitStack() as ctx:
        pstat = ctx.enter_context(tc.tile_pool(name="stat", bufs=1))
        pidx = ctx.enter_context(tc.tile_pool(name="pidx", bufs=2))
        pg = ctx.enter_context(tc.tile_pool(name="pg", bufs=pg_bufs))
        pone = ctx.enter_context(tc.tile_pool(name="pone", bufs=6))
        psm = ctx.enter_context(tc.tile_pool(name="psm", bufs=3))
        pnode = ctx.enter_context(tc.tile_pool(name="pnode", bufs=3))
        pps_agg = ctx.enter_context(tc.tile_pool(name="ppsagg", bufs=ps_bufs, space="PSUM"))
        pps_t = ctx.enter_context(tc.tile_pool(name="ppst", bufs=2, space="PSUM"))
        pps_h = ctx.enter_context(tc.tile_pool(name="ppsh", bufs=2, space="PSUM"))

        iota_s = pstat.tile([128, 128], bf16, tag="iota")
        nc.sync.dma_start(out=iota_s[:], in_=d_iota[:])
        ident_s = pstat.tile([128, 128], bf16, tag="ident")
        nc.sync.dma_start(out=ident_s[:], in_=d_ident[:])
        wl1_s = pstat.tile([128, 128], bf16, tag="wl1")
        nc.sync.dma_start(out=wl1_s[:], in_=d_wl1[:])
        wr1_s = pstat.tile([128, 128], bf16, tag="wr1")
        nc.sync.dma_start(out=wr1_s[:], in_=d_wr1[:])
        wl2_s = pstat.tile([128, 128], bf16, tag="wl2")
        nc.sync.dma_start(out=wl2_s[:], in_=d_wl2[:])
        wr2_s = pstat.tile([128, 128], bf16, tag="wr2")
        nc.sync.dma_start(out=wr2_s[:], in_=d_wr2[:])
        b1_s = pstat.tile([128, 1], f32, tag="b1")
        nc.sync.dma_start(out=b1_s[:], in_=d_b1[:])
        b2_s = pstat.tile([128, 1], f32, tag="b2")
        nc.sync.dma_start(out=b2_s[:], in_=d_b2[:])
        dstloc_s = pstat.tile([128, NCH], f32, tag="dstloc")
        nc.sync.dma_start(out=dstloc_s[:], in_=d_dstloc[:])
        rdeg_s = pstat.tile([128, NCH], f32, tag="rdeg")
        nc.sync.dma_start(out=rdeg_s[:], in_=d_rdeg[:])
        xT_s = pstat.tile([128, NPCP], bf16, tag="xT")
        nc.sync.dma_start(out=xT_s[:], in_=x_selfT[:])
        hT_all = pstat.tile([128, NPCP], bf16, tag="hT")

        self_qn = [0]

        def emit_body(do_ag=True):
            for layer in (1, 2):
                if layer == 1:
                    LP = L1
                    lo_ap = x_tab[0:TAB1, :]
                    hi_ap = x_tab[TAB1:N, :]
                    d_lo, d_hi = d_idx1_lo, d_idx1_hi
                    wl_s, wr_s, bias_s = wl1_s, wr1_s, b1_s
                    q = 0
                else:
                    LP = L2
                    lo_ap = h_full[0:TAB2, :]
                    hi_ap = h_full[TAB2:NP, :]
                    d_lo, d_hi = d_idx2_lo, d_idx2_hi
                    wl_s, wr_s, bias_s = wl2_s, wr2_s, b2_s
                    q = L1["NCH"]
                c_lo, c_hi = LP["c_lo"], LP["c_hi"]
                nch = [c_lo[b] + c_hi[b] for b in range(NB)]
                sbs = [list(range(s, min(s + SBS, NB))) for s in range(0, NB, SBS)]

                off_lo = 0
                off_hi = 0
                for sb in sbs:
                    nlo = sum(c_lo[b] for b in sb) * 128
                    nhi = sum(c_hi[b] for b in sb) * 128
                    tlo = pidx.tile([128, nlo // 16], i16, tag="idxlo")
                    nc.sync.dma_start(out=tlo[:], in_=d_lo[:, off_lo // 16:(off_lo + nlo) // 16])
                    thi = pidx.tile([128, nhi // 16], i16, tag="idxhi")
                    nc.sync.dma_start(out=thi[:], in_=d_hi[:, off_hi // 16:(off_hi + nhi) // 16])

                    # Rotate desc-gen calls over all 4 SWDGE queues; each call
                    # is capped by the per-queue descriptor FIFO capacity.
                    def qpick(side):
                        if one_queue:
                            # CoreSim locks each tile DMA sem to one queue;
                            # timing there is queue-independent anyway.
                            return 0
                        self_qn[0] += 1
                        return self_qn[0] % 4
                    gm = gmax or GMAX
                    g_lo = pg.tile([128, nlo // 128, 128], bf16, tag="glo")
                    for o in range(0, nlo, gm):
                        nn = min(gm, nlo - o)
                        nc.gpsimd.dma_gather(
                            out_ap=g_lo[:, o // 128:(o + nn) // 128, :], in_ap=lo_ap,
                            idxs_ap=tlo[:, o // 16:(o + nn) // 16],
                            num_idxs=nn, num_idxs_reg=nn, elem_size=F,
                            single_packet=sp, queue_num=qpick(0))
                    g_hi = pg.tile([128, nhi // 128, 128], bf16, tag="ghi")
                    for o in range(0, nhi, gm):
                        nn = min(gm, nhi - o)
                        nc.gpsimd.dma_gather(
                            out_ap=g_hi[:, o // 128:(o + nn) // 128, :], in_ap=hi_ap,
                            idxs_ap=thi[:, o // 16:(o + nn) // 16],
                            num_idxs=nn, num_idxs_reg=nn, elem_size=F,
                            single_packet=sp, queue_num=qpick(1))
                    off_lo += nlo
                    off_hi += nhi

                    if stage == "gather":
                        q += sum(nch[b] for b in sb)
                        continue
                    col_lo = 0
                    col_hi = 0
                    ps_blocks = {}
                    for b in sb:
                        # all SBS blocks' aggregation matmuls are emitted
                        # before any tail so the in-order PE queue doesn't
                        # stall the next block's aggregation behind DVE/ACT
                        # tail work
                        ps_agg = pps_agg.tile([128, 128], f32, tag="psagg")
                        ps_blocks[b] = ps_agg
                        for j in range(nch[b]):
                            onehot = pone.tile([128, 128], bf16, tag="onehot")
                            nc.vector.tensor_scalar(
                                onehot[:], iota_s[:], dstloc_s[:, q:q + 1],
                                rdeg_s[:, q:q + 1], OP.is_equal, OP.mult)
                            if j < c_lo[b]:
                                rhs = g_lo[:, col_lo, :]
                                col_lo += 1
                            else:
                                rhs = g_hi[:, col_hi, :]
                                col_hi += 1
                            # psum_aggT[f, v] += sum_e g[e, f] * onehot[e, v]
                            nc.tensor.matmul(
                                ps_agg[:], rhs, onehot[:],
                                start=(j == 0), stop=(j == nch[b] - 1))
                            q += 1
                    if stage == "agg":
                        continue
                    for b in sb:
                        aggT = psm.tile([128, 128], bf16, tag="aggT")
                        nc.scalar.copy(aggT[:], ps_blocks[b][:])
                        if layer == 1:
                            rhs2 = xT_s[:, b * 128:(b + 1) * 128]
                        else:
                            rhs2 = hT_all[:, b * 128:(b + 1) * 128]
                        ps_h = pps_h.tile([128, 128], f32, tag="psh")
                        nc.tensor.matmul(ps_h[:], wl_s[:], aggT[:], start=True, stop=False)
                        nc.tensor.matmul(ps_h[:], wr_s[:], rhs2, start=False, stop=True)
                        if layer == 1:
                            hT_blk = hT_all[:, b * 128:(b + 1) * 128]
                            nc.scalar.activation(hT_blk, ps_h[:], AF.Relu, bias=bias_s[:])
                            ps_t = pps_t.tile([128, 128], bf16, tag="pst")
                            nc.tensor.transpose(ps_t[:], hT_blk, ident_s[:])
                            nodeb = pnode.tile([128, 128], bf16, tag="nodeb")
                            nc.scalar.copy(nodeb[:], ps_t[:])
                            nc.sync.dma_start(
                                out=h_shard[b * 128:(b + 1) * 128, :], in_=nodeb[:])
                        else:
                            oT = pnode.tile([128, 128], f32, tag="oT")
                            nc.scalar.activation(oT[:], ps_h[:], AF.Relu, bias=bias_s[:])
                            nc.sync.dma_start(
                                out=out_shardT[:, b * 128:(b + 1) * 128], in_=oT[:])

                if layer == 1 and do_ag:
                    nc.gpsimd.collective_compute(
                        "AllGather", OP.bypass, replica_groups=[list(range(NC))],
                        ins=[h_shard[:]], outs=[h_full[:]])

        if loop_reps:
            # timing variant: collectives can't live inside control flow;
            # init h_full once and loop the 2-layer pipeline
            nc.gpsimd.collective_compute(
                "AllGather", OP.bypass, replica_groups=[list(range(NC))],
                ins=[h_shard[:]], outs=[h_full[:]])
            with tc.For_i(0, loop_reps, 1):
                emit_body(do_ag=False)
        else:
            emit_body(do_ag=True)
    nc.compile()
    return nc


def _get_program(prep, loop_reps=0, stage="full", one_queue=False, gmax=None,
                 pg_bufs=2, ps_bufs=4, sp=True):
    key = (prep["L1"]["c_lo"], prep["L1"]["c_hi"],
           prep["L2"]["c_lo"], prep["L2"]["c_hi"], loop_reps, stage, one_queue, gmax,
           pg_bufs, ps_bufs, sp)
    if key not in _cache:
        _cache[key] = _build(prep["L1"], prep["L2"], prep["NCH"], loop_reps, stage,
                             one_queue, gmax, pg_bufs, ps_bufs, sp)
    return _cache[key]


def _in_maps(prep, x, W1_l, b1, W1_r, W2_l, b2, W2_r):
    bf16 = _bf16()
    x = np.asarray(x, np.float32)
    xb = np.ascontiguousarray(x.astype(bf16))
    iota = np.ascontiguousarray(
        np.broadcast_to(np.arange(128, dtype=np.float32), (128, 128))).astype(bf16)
    ident = np.eye(128, dtype=np.float32).astype(bf16)
    common = {
        "x_tab": xb,
        "wl1": np.ascontiguousarray(np.asarray(W1_l, np.float32)).astype(bf16),
        "wr1": np.ascontiguousarray(np.asarray(W1_r, np.float32)).astype(bf16),
        "wl2": np.ascontiguousarray(np.asarray(W2_l, np.float32)).astype(bf16),
        "wr2": np.ascontiguousarray(np.asarray(W2_r, np.float32)).astype(bf16),
        "b1c": np.ascontiguousarray(np.asarray(b1, np.float32).reshape(128, 1)),
        "b2c": np.ascontiguousarray(np.asarray(b2, np.float32).reshape(128, 1)),
        "iota": iota,
        "ident": ident,
    }
    maps = []
    for c in range(NC):
        xsT = np.zeros((128, NPCP), np.float32)
        xsT[:, :NPC] = x[c * NPC:(c + 1) * NPC].T
        m = dict(common)
        m["x_selfT"] = np.ascontiguousarray(xsT.astype(bf16))
        m["idx1_lo"] = np.ascontiguousarray(prep["L1"]["idx_lo"][c])
        m["idx1_hi"] = np.ascontiguousarray(prep["L1"]["idx_hi"][c])
        m["idx2_lo"] = np.ascontiguousarray(prep["L2"]["idx_lo"][c])
        m["idx2_hi"] = np.ascontiguousarray(prep["L2"]["idx_hi"][c])
        m["dstloc"] = np.ascontiguousarray(prep["dstloc"][c])
        m["rdeg"] = np.ascontiguousarray(prep["rdeg"][c])
        maps.append(m)
    return maps


def kernel(x, edge_index, W1_l, b1, W1_r, W2_l, b2, W2_r):
    from concourse.bass_utils import run_bass_kernel_spmd

    x = np.asarray(x, np.float32)
    assert x.shape == (N, F) and np.asarray(edge_index).shape == (2, E)
    prep = _host_prep(x, edge_index)
    nc = _get_program(prep)
    maps = _in_maps(prep, x, W1_l, b1, W1_r, W2_l, b2, W2_r)
    res = run_bass_kernel_spmd(nc, maps, list(range(NC)))
    out = np.concatenate(
        [np.asarray(res.results[c]["out_shardT"]).T[:NPC] for c in range(NC)], axis=0)
    return np.ascontiguousarray(out.astype(np.float32))

